# revision 1
# baseline (speedup 1.0000x reference)
"""Attention-Augmented Conv2D (AAConv2D) distributed Bass kernel for 8 TRN2 NeuronCores.

Strategy: pure data-parallel over batch (B=8 -> one image per core, weights
replicated, zero collectives). Per core, for one [32,32,256] image:

  conv branch : 3x3 SAME conv (256->256ch) as 9 shifted-window matmuls
                accumulated in PSUM; its matmuls are interleaved into the
                attention head loop as PE filler (the loop is ACT-bound).
  attn branch : kqv 1x1 conv (channel-major k/q, position-major v),
                per-head S^T = K Q^T computed with an AUGMENTED contraction
                (32 qk dims + 64 one-hot w/h-offset dims = 96) so the
                relative-position logits ride in the same matmul stream;
                exp on ScalarE (max-free softmax, logits are O(10) so fp32
                exp is safe); P^T V via matmul with [v|1] stationary
                (sumexp ride-along); per-head normalize on VectorE/GpSimd;
                output projection.

Schedule notes (v2):
  - ScalarE exp is the critical engine (~64 x 1.33us). Everything else is
    arranged to hide under it: conv matmuls are popped into per-head slots,
    rel-logit matmuls for head h+2 are injected into the S psum pool's
    round-robin, all per-head stationaries/rhs rows are built by a handful
    of big broadcast DMAs right after the kqv escapes (no per-head DMA
    latency on the critical path).
  - A short memset-fed prewarm bridges the framework preamble so the HAM
    clock-gate is at 2.4 GHz when the first real matmul issues.
  - PSUM budget (8 banks): S pool 2x[128,1024] (4) + PV accum (2) +
    conv accum (2).

All heavy matmuls in bf16 (fp32 matmul is 4x slower on TRN2 PE).
Host does layout-only prep: batch sharding, transposes to channel-major,
bf16 casts, relative-table window expansion, one-hot delta tables, and the
exact algebraic fold of the v-bias into the projection bias.
"""

import contextlib

import numpy as np
import ml_dtypes

BF16 = ml_dtypes.bfloat16

B, H, W, FIN = 8, 32, 32, 256
POS = H * W
FOUT, K, DK, DV, NH = 512, 3, 256, 256, 8
DKH, DVH = DK // NH, DV // NH
FOUT_CONV = FOUT - DV  # 256
N_CORES = 8

_PROG_CACHE = {}


def _build_program(variant="full"):
    """Build (and cache) the compiled Bass program. Same program for all 8
    cores (SPMD); per-core data arrives via the per-core input maps."""
    if ("nc", variant) in _PROG_CACHE:
        return _PROG_CACHE[("nc", variant)]

    import concourse.bass as bass
    import concourse.bacc as bacc
    import concourse.tile as tile
    from concourse import mybir

    BF = mybir.dt.bfloat16
    F32 = mybir.dt.float32
    EXP = mybir.ActivationFunctionType.Exp

    nc = bacc.Bacc("TRN2", target_bir_lowering=False, debug=False,
                   num_devices=N_CORES)

    # ---- DRAM parameters ----
    PADW = H + 2  # 34
    xpad_d = nc.dram_tensor("xpad", [FIN, PADW * PADW], BF, kind="ExternalInput")
    xtc_d = nc.dram_tensor("xtc", [FIN, POS], BF, kind="ExternalInput")
    wkqv_d = nc.dram_tensor("wkqv", [FIN, 2 * DK + DV], BF, kind="ExternalInput")
    wconv_d = nc.dram_tensor("wconv", [2, 128, K * K * FOUT_CONV], BF,
                             kind="ExternalInput")
    wproj_d = nc.dram_tensor("wproj", [DV, DV], BF, kind="ExternalInput")
    biases_d = nc.dram_tensor("biases", [128, 8], F32, kind="ExternalInput")
    krw_d = nc.dram_tensor("krw4", [128, 1024], BF, kind="ExternalInput")
    krh_d = nc.dram_tensor("krh4", [128, 1024], BF, kind="ExternalInput")
    delta_d = nc.dram_tensor("delta", [2 * W, POS], BF, kind="ExternalInput")
    out_d = nc.dram_tensor("out", [FOUT, POS], BF, kind="ExternalOutput")

    xpad_in = xpad_d.ap()
    xtc = xtc_d.ap()
    wkqv = wkqv_d.ap()
    wconv = wconv_d.ap()
    wproj = wproj_d.ap()
    biases = biases_d.ap()
    krw = krw_d.ap()
    krh = krh_d.ap()
    delta = delta_d.ap()
    out = out_d.ap()

    with tile.TileContext(nc) as tc, contextlib.ExitStack() as ctx:
        consts = ctx.enter_context(tc.tile_pool(name="consts", bufs=1))
        xpads = ctx.enter_context(tc.tile_pool(name="xpads", bufs=1))
        kqpool = ctx.enter_context(tc.tile_pool(name="kqpool", bufs=1))
        vopool = ctx.enter_context(tc.tile_pool(name="vopool", bufs=1))
        attall = ctx.enter_context(tc.tile_pool(name="attall", bufs=1))
        megas = ctx.enter_context(tc.tile_pool(name="megas", bufs=1))
        cacc = ctx.enter_context(tc.tile_pool(name="cacc", bufs=1))
        work = ctx.enter_context(tc.tile_pool(name="work", bufs=4))
        small = ctx.enter_context(tc.tile_pool(name="small", bufs=4))
        outp = ctx.enter_context(tc.tile_pool(name="outp", bufs=2))
        # PSUM: 8 banks. ps_s 3x[128,1024]f32 = 6 banks (S chunks, kqv, v,
        # conv groups, proj, prewarm all rotate through it), ps_at 2 banks
        # (PV accumulator rows 0:33 + rel scratch rows 64:128).
        ps_s = ctx.enter_context(tc.tile_pool(name="ps_s", bufs=3,
                                              space=bass.MemorySpace.PSUM))
        ps_at = ctx.enter_context(tc.tile_pool(name="ps_at", bufs=1,
                                               space=bass.MemorySpace.PSUM))

        # ---- startup input DMAs. sync queue: kqv-critical consts, then
        # head-0/1 preps. scalar queue: conv/proj weights, then head-2..7
        # preps (each lands well before its rel/S consumer). ----
        xt_sb = []
        for f in range(2):
            tu = xpads.tile([128, POS], BF, tag=f"xtsb{f}")
            nc.sync.dma_start(out=tu[:], in_=xtc[f * 128:(f + 1) * 128, :])
            xt_sb.append(tu)
        wkqv_sb = []
        for f in range(2):
            t = consts.tile([128, 2 * DK + DV], BF, tag=f"wkqv{f}")
            nc.sync.dma_start(out=t[:], in_=wkqv[f * 128:(f + 1) * 128, :])
            wkqv_sb.append(t)
        krw_sb = consts.tile([128, 1024], BF, tag="krw")
        nc.sync.dma_start(out=krw_sb[:], in_=krw[:, :])
        krh_sb = consts.tile([128, 1024], BF, tag="krh")
        nc.sync.dma_start(out=krh_sb[:], in_=krh[:, :])
        ball_sb = consts.tile([128, 8], F32, tag="ball")
        nc.sync.dma_start(out=ball_sb[:], in_=biases[:, :])
        bkq_sb = [ball_sb[:, cc:cc + 1] for cc in range(4)]
        bconv_sb = [ball_sb[:, 4 + co:5 + co] for co in range(2)]
        bproj_sb = [ball_sb[:, 6 + co:7 + co] for co in range(2)]

        stmega = megas.tile([128, 8 * POS], BF, tag="stmega")
        st3 = stmega.rearrange("p (h c) -> p h c", h=8)
        nc.scalar.dma_start(
            out=st3[32:96, :, :],
            in_=delta[:, :].unsqueeze(1).broadcast_to([2 * W, 8, POS]))
        xpad = []
        for f in range(2):
            t = xpads.tile([128, PADW * PADW], BF, tag=f"xpad{f}")
            nc.scalar.dma_start(out=t[:], in_=xpad_in[f * 128:(f + 1) * 128, :])
            xpad.append(t)
        wconv_sb = []
        for f in range(2):
            t = consts.tile([128, K * K * FOUT_CONV], BF, tag=f"wconv{f}")
            nc.scalar.dma_start(out=t[:], in_=wconv[f, :, :])
            wconv_sb.append(t)
        wproj_sb = []
        for f in range(2):
            t = consts.tile([128, DV], BF, tag=f"wproj{f}")
            nc.scalar.dma_start(out=t[:], in_=wproj[f * 128:(f + 1) * 128, :])
            wproj_sb.append(t)

        rhmega = megas.tile([128, 8 * POS], BF, tag="rhmega")
        rh3 = rhmega.rearrange("p (h c) -> p h c", h=8)
        # q replicas for the row-tiled rel matmuls (all 4 strips: rel_w
        # runs in row strips 0/1, rel_h in 2/3, quads fully concurrent)
        qrepm = megas.tile([128, 8 * POS], BF, tag="qrepm")
        qrep4 = qrepm.rearrange("(s p) (h c) -> p s h c", s=4, h=8)
        ones_sb = consts.tile([1, 32], BF, tag="ones")
        nc.vector.memset(ones_sb[:], 1.0)

        # ---- PE pre-warm: bridge the framework preamble so the HAM
        # clock-gate is released before the first real matmul ----
        wu = consts.tile([128, 512], BF, tag="wu")
        nc.vector.memset(wu[:], 0.25)
        wups = ps_s.tile([128, POS], F32, tag="ps", name="wups")
        for i in range(12):
            nc.tensor.matmul(wups[:, 0:512], lhsT=wu[:, 0:128], rhs=wu[:],
                             start=True, stop=True)

        def xwin(f, dy, dx, h0, hn):
            # [128, hn, 32] window of the padded image
            t3 = xpad[f].rearrange("p (a b) -> p a b", a=PADW)
            return t3[:, h0 + dy:h0 + dy + hn, dx:dx + W]

        # ---- kqv: k and q sections, channel-major [co, pos] ----
        # order (q0, k0, q1, k1); per-head prep copies emitted per section:
        # heads 0/1 on the sync queue (earliest consumers), 2-7 on scalar
        kq_sb = [None] * 4
        for cc in (2, 0, 3, 1):
            ps = ps_s.tile([128, POS], F32, tag="ps", name=f"kqps{cc}")
            for f in range(2):
                for nh in range(2):
                    nc.tensor.matmul(
                        ps[:, nh * 512:(nh + 1) * 512],
                        lhsT=wkqv_sb[f][:, cc * 128:(cc + 1) * 128],
                        rhs=xt_sb[f][:, nh * 512:(nh + 1) * 512],
                        start=(f == 0), stop=(f == 1))
            t = kqpool.tile([128, POS], BF, tag=f"kq{cc}", name=f"kq{cc}")
            nc.vector.tensor_scalar_add(out=t[:], in0=ps[:], scalar1=bkq_sb[cc][:])
            kq_sb[cc] = t
            sec = cc % 2  # head group of this section (0: h0-3, 1: h4-7)
            for hh in range(4):
                h = 4 * sec + hh
                eng = nc.sync if h in (0, 1) else nc.scalar
                src = t[32 * hh:32 * hh + 32, :]
                if cc >= 2:  # q section: rel q replicas + rhs q rows
                    for s in range(4):
                        eng.dma_start(out=qrep4[:, s, h, :], in_=src)
                    eng.dma_start(out=rh3[0:32, h, :], in_=src)
                else:        # k section: stationary k rows
                    eng.dma_start(out=st3[0:32, h, :], in_=src)

        # ---- v: position-major [pos, dv] -> vomega with ones interleave ----
        vomega = vopool.tile([128, 8 * NH * (DVH + 1)], BF, tag="vomega")
        vom4 = vomega.rearrange("p (k h d) -> p k h d", k=8, d=DVH + 1)
        nc.vector.memset(vom4[:, :, :, DVH:DVH + 1], 1.0)
        for half in range(2):
            ps = ps_s.tile([128, POS], F32, tag="ps", name=f"vps{half}")
            for q in range(4):
                kc = half * 4 + q
                for f in range(2):
                    nc.tensor.matmul(
                        ps[:, q * 256:(q + 1) * 256],
                        lhsT=xt_sb[f][:, kc * 128:(kc + 1) * 128],
                        rhs=wkqv_sb[f][:, 2 * DK:2 * DK + DV],
                        start=(f == 0), stop=(f == 1))
            nc.vector.tensor_copy(
                out=vom4[:, half * 4:(half + 1) * 4, :, 0:DVH],
                in_=ps.rearrange("p (k h d) -> p k h d", k=4, d=DVH))

        att_all = []
        for f in range(2):
            t = attall.tile([128, POS], BF, tag=f"att{f}", name=f"att{f}")
            att_all.append(t)
        # conv fp32 SBUF accumulators (groups of 9 psum matmuls are folded
        # in with DVE adds — this keeps conv pinned inside the head loop
        # via the ps_s rotation, and frees 2 psum banks for ps_s bufs=3)
        conv_acc = []
        for co in range(2):
            t = cacc.tile([128, POS], F32, tag=f"cacc{co}")
            nc.vector.memset(t[:], 0.0)
            conv_acc.append(t)
        proj0_sb = []
        for co in range(2):
            t = cacc.tile([128, POS], F32, tag=f"proj0{co}", name=f"proj0{co}")
            proj0_sb.append(t)

        # ---- rel-logit matmuls for head h, escaping to rhmega rows 32:96.
        # rp is a [128, POS] psum tile; rel_w lands at rows row0:row0+32,
        # rel_h at rows row0+32:row0+64 (in the steady loop rp is the PV
        # accumulator tile of the RUNNING head, whose partitions 64:128 are
        # unused — rel rides in those for free, costing no psum banks).
        # skip_check: the sim's psum group-check shadow mis-indexes
        # partition bases and false-fires on the cohabitation (HW
        # has_written is per-element; the sim DATA model indexes partitions
        # correctly, so the numeric check still validates the scheme).
        def rel_head(h, rp, row0, skip_check=False):
            # 4-way concurrent quads: rel_w in row strips 0/1 (different
            # banks for the (a, a+16) pair), rel_h in row strips 2/3.
            # rel_w: for wq = 16r + a, out block [w', hq] at psum col
            # 512r + 32a; rel_h: for hq = 16r + a, out block [h', wq]
            # directly at col 32*hq (contiguous).
            qr3 = qrep4[:, :, h, :].rearrange("p s (b a) -> p s a b", a=W)
            for a in range(16):
                for r in range(2):
                    wq = 16 * r + a
                    nc.tensor.matmul(
                        rp[row0:row0 + 32,
                           512 * r + 32 * a:512 * r + 32 * a + 32],
                        lhsT=krw_sb[32 * r:32 * r + 32, wq * 32:(wq + 1) * 32],
                        rhs=qr3[:, r, wq, :],
                        start=True, stop=True, tile_position=(32 * r, row0),
                        skip_group_check=skip_check)
                for r in range(2):
                    hq = 16 * r + a
                    nc.tensor.matmul(
                        rp[row0 + 32:row0 + 64, 32 * hq:32 * hq + 32],
                        lhsT=krh_sb[64 + 32 * r:64 + 32 * r + 32,
                                    hq * 32:(hq + 1) * 32],
                        rhs=qrep4[:, 2 + r, h, hq * 32:(hq + 1) * 32],
                        start=True, stop=True,
                        tile_position=(64 + 32 * r, row0 + 32),
                        skip_group_check=skip_check)
            # escapes: un-permute rel_w (src col = 512r + 32a, dst 32*hq+wq,
            # wq = 16r + a), straight-copy rel_h
            rw_src = rp[row0:row0 + 32, :].rearrange(
                "p (r a h) -> p r h a", r=2, a=16)
            rw_dst = rh3[32:64, h, :].rearrange(
                "p (h r a) -> p r h a", r=2, a=16)
            for r in range(2):
                nc.vector.tensor_copy(out=rw_dst[:, r], in_=rw_src[:, r])
            nc.vector.tensor_copy(out=rh3[64:96, h, :],
                                  in_=rp[row0 + 32:row0 + 64, :])

        # ---- conv group for head h: 9 tap-matmuls of one (co, nh, f)
        # cell into a ps_s tile, then DVE-add into the fp32 accumulator ----
        def conv_group(h):
            co, nh, f = h // 4, (h // 2) % 2, h % 2
            ps = ps_s.tile([128, POS], F32, tag="ps", name=f"cps{h}")
            for tp in range(9):
                dy, dx = tp // 3, tp % 3
                o0 = tp * FOUT_CONV + co * 128
                nc.tensor.matmul(
                    ps[:, nh * 512:(nh + 1) * 512],
                    lhsT=wconv_sb[f][:, o0:o0 + 128],
                    rhs=xwin(f, dy, dx, nh * 16, 16),
                    start=(tp == 0), stop=(tp == 8))
            acc = conv_acc[co][:, nh * 512:(nh + 1) * 512]
            nc.vector.tensor_add(acc, acc, ps[:, nh * 512:(nh + 1) * 512])

        def conv_escape(co):
            ot = outp.tile([128, POS], BF, tag="out", name=f"cot{co}")
            nc.vector.tensor_scalar_add(out=ot[:], in0=conv_acc[co][:],
                                        scalar1=bconv_sb[co][:])
            nc.sync.dma_start(out=out[co * 128:(co + 1) * 128, :], in_=ot[:])

        def proj_half(f):
            # f=0 part (heads 0-3) runs right after head 3; f=1 plus the
            # combine runs in the tail
            for co in range(2):
                ps = ps_s.tile([128, POS], F32, tag="ps", name=f"pps{f}{co}")
                for nh in range(2):
                    nc.tensor.matmul(
                        ps[:, nh * 512:(nh + 1) * 512],
                        lhsT=wproj_sb[f][:, co * 128:(co + 1) * 128],
                        rhs=att_all[f][:, nh * 512:(nh + 1) * 512],
                        start=True, stop=True)
                if f == 0:
                    nc.vector.tensor_copy(out=proj0_sb[co][:], in_=ps[:])
                else:
                    ot = outp.tile([128, POS], BF, tag="out", name=f"pot{co}")
                    nc.vector.scalar_tensor_tensor(
                        out=ot[:], in0=ps[:], scalar=bproj_sb[co][:],
                        in1=proj0_sb[co][:],
                        op0=mybir.AluOpType.add, op1=mybir.AluOpType.add)
                    nc.sync.dma_start(
                        out=out[FOUT_CONV + co * 128:FOUT_CONV + (co + 1) * 128, :],
                        in_=ot[:])

        # ---- per-head attention ----
        def inner_head(h):
            # full-height tile: rows 0:DVH+1 = PV accumulator, rows 64:128
            # are scratch for head h+2's rel matmuls (no extra psum banks)
            at = ps_at.tile([128, POS], F32, tag="at", name=f"at{h}")
            psb_t = [None] * 8

            def s_step(kc):
                sps = ps_s.tile([128, POS], F32, tag="ps", name=f"sps{h}_{kc}")
                for nh in range(2):
                    nc.tensor.matmul(
                        sps[:, nh * 512:(nh + 1) * 512],
                        lhsT=stmega[0:96, h * POS + kc * 128:
                                    h * POS + (kc + 1) * 128],
                        rhs=rhmega[0:96, h * POS + nh * 512:
                                   h * POS + (nh + 1) * 512],
                        start=True, stop=True)
                psb = work.tile([128, POS], BF, tag="pexp", name=f"psb{h}_{kc}")
                nc.scalar.activation(out=psb[:], in_=sps[:], func=EXP)
                psb_t[kc] = psb

            def pv_step(kc):
                for nh in range(2):
                    nc.tensor.matmul(
                        at[0:DVH + 1, nh * 512:(nh + 1) * 512],
                        lhsT=vomega[:, kc * NH * (DVH + 1) + h * (DVH + 1):
                                    kc * NH * (DVH + 1) + (h + 1) * (DVH + 1)],
                        rhs=psb_t[kc][:, nh * 512:(nh + 1) * 512],
                        start=(kc == 0), stop=(kc == 7),
                        skip_group_check=True)

            s_step(0)
            s_step(1)
            pv_step(0)
            for kc in range(2, 8):
                s_step(kc)
                pv_step(kc - 1)
            pv_step(7)

            # psum-escape copy first (frees rows 0:33 for the next head),
            # then the conv group and head h+2's rel matmuls as PE filler
            # under the ACT drain of this head's last exps
            cmb = small.tile([DVH + 1, POS], BF, tag="cmb", name=f"cmb{h}")
            nc.vector.tensor_copy(out=cmb[:], in_=at[0:DVH + 1, :])
            conv_group(h)
            if h + 2 < 8:
                rel_head(h + 2, at, 64, skip_check=True)

            # normalize: attn_h = (P^T V)[0:32] / sumexp (row 32)
            sec = h // 4
            g = (h % 4) * 32
            s8 = small.tile([128, 8], BF, tag="s8", name=f"s8{h}")
            nc.sync.dma_start(out=s8[:], in_=cmb[DVH:DVH + 1, :])
            rcp8 = small.tile([128, 8], BF, tag="rcp8", name=f"rcp8{h}")
            with nc.allow_low_precision(reason="1/sumexp in bf16 is within "
                                        "the softmax rounding budget"):
                nc.vector.reciprocal(out=rcp8[:], in_=s8[:])
            rcpf = small.tile([1, POS], BF, tag="rcpf", name=f"rcpf{h}")
            nc.sync.dma_start(out=rcpf[:], in_=rcp8[:])
            an = small.tile([32, POS], BF, tag="an", name=f"an{h}")
            if h == 7:
                # tail: broadcast 1/sumexp via a K=1 PE matmul (PE is idle
                # here and this cuts two GpSimd queue hops off the tail)
                rps = ps_s.tile([128, POS], F32, tag="ps", name="rcppe")
                for nh in range(2):
                    nc.tensor.matmul(
                        rps[0:32, nh * 512:(nh + 1) * 512],
                        lhsT=ones_sb[:, :],
                        rhs=rcpf[:, nh * 512:(nh + 1) * 512],
                        start=True, stop=True)
                nc.vector.tensor_mul(an[:], cmb[0:DVH, :], rps[0:32, :])
            else:
                rcpb = small.tile([32, POS], BF, tag="rcpb", name=f"rcpb{h}")
                nc.gpsimd.partition_broadcast(rcpb[:], rcpf[:])
                nc.vector.tensor_mul(an[:], cmb[0:DVH, :], rcpb[:])
            nc.gpsimd.dma_start(out=att_all[sec][g:g + 32, :], in_=an[:])

        # prime the first two heads' rel tables (pre-loop: psum from the
        # ps_s rotation, rows 32:96 — no ACT contention yet), then the loop
        rel_head(0, ps_s.tile([128, POS], F32, tag="ps", name="rp0"), 32)
        rel_head(1, ps_s.tile([128, POS], F32, tag="ps", name="rp1"), 32)
        for h in range(8):
            inner_head(h)
            if h == 3:
                conv_escape(0)
                proj_half(0)
        conv_escape(1)
        proj_half(1)

    nc.compile()
    _PROG_CACHE[("nc", variant)] = nc
    return nc


def _host_prep(x, w_kqv, b_kqv, w_proj, b_proj, w_conv, b_conv,
               key_rel_w, key_rel_h):
    """Layout-only host prep -> per-core input maps."""
    x = np.asarray(x, np.float32)
    w_kqv = np.asarray(w_kqv, np.float32)
    b_kqv = np.asarray(b_kqv, np.float32)
    w_proj = np.asarray(w_proj, np.float32)
    b_proj = np.asarray(b_proj, np.float32)
    w_conv = np.asarray(w_conv, np.float32)
    b_conv = np.asarray(b_conv, np.float32)
    key_rel_w = np.asarray(key_rel_w, np.float32)
    key_rel_h = np.asarray(key_rel_h, np.float32)

    scale = np.float32(DKH ** -0.5)
    wkqv = w_kqv.copy()
    wkqv[:, DK:2 * DK] *= scale           # fold q scaling into the weights
    bkq = b_kqv[:2 * DK].copy()
    bkq[DK:] *= scale
    # fold the v bias through the projection: attn = (attn0 + bv) Wp + bp
    bproj_eff = b_proj + b_kqv[2 * DK:] @ w_proj
    # combined per-partition bias tile [128, 8]:
    # cols 0-3 = b_kq 128-chunks, 4-5 = b_conv chunks, 6-7 = b_proj chunks
    ball = np.stack([bkq[0:128], bkq[128:256], bkq[256:384], bkq[384:512],
                     b_conv[0:128], b_conv[128:256],
                     bproj_eff[0:128], bproj_eff[128:256]], axis=1)

    # window-expanded relative tables, replicated to all 4 partition groups:
    #   krw4[32r + d, wq*32 + w'] = key_rel_w[w' - wq + 31, d]
    idx = (np.arange(W)[None, :] - np.arange(W)[:, None] + (W - 1))  # [wq, w']
    krw = key_rel_w[idx]                   # [wq, w', 32]
    krw4 = np.tile(krw.transpose(2, 0, 1).reshape(DKH, W * W), (4, 1))
    krh = key_rel_h[idx]
    krh4 = np.tile(krh.transpose(2, 0, 1).reshape(DKH, H * H), (4, 1))

    # one-hot offset deltas: rows 0-31 wk one-hots, rows 32-63 hk one-hots
    kpos = np.arange(POS)
    deltas = np.zeros((2 * W, POS), np.float32)
    deltas[kpos % W, kpos] = 1.0
    deltas[W + kpos // W, kpos] = 1.0

    # conv weights repacked so each 128-channel chunk's 9 taps are one
    # contiguous per-partition run: wconv[f][p, tp*256 + o]
    wc = w_conv.reshape(K * K, 2, 128, FOUT_CONV)          # [tap, f, p, o]
    wc = np.ascontiguousarray(wc.transpose(1, 2, 0, 3)).reshape(
        2, 128, K * K * FOUT_CONV)

    shared = {
        "wkqv": wkqv.astype(BF16),
        "wconv": wc.astype(BF16),
        "wproj": w_proj.astype(BF16),
        "biases": ball.astype(np.float32),
        "krw4": krw4.astype(BF16),
        "krh4": krh4.astype(BF16),
        "delta": deltas.astype(BF16),
    }
    PADW = H + 2
    in_maps = []
    for b in range(N_CORES):
        m = dict(shared)
        xt = np.ascontiguousarray(x[b].reshape(POS, FIN).T)   # [FIN, POS]
        xp = np.zeros((FIN, PADW, PADW), np.float32)
        xp[:, 1:H + 1, 1:W + 1] = xt.reshape(FIN, H, W)
        m["xpad"] = xp.reshape(FIN, PADW * PADW).astype(BF16)
        m["xtc"] = xt.astype(BF16)
        in_maps.append(m)
    return in_maps


def kernel(x, w_kqv, b_kqv, w_proj, b_proj, w_conv, b_conv,
           key_rel_w, key_rel_h):
    from concourse.bass_utils import run_bass_kernel_spmd

    nc = _build_program()
    in_maps = _host_prep(x, w_kqv, b_kqv, w_proj, b_proj, w_conv, b_conv,
                         key_rel_w, key_rel_h)
    if not _PROG_CACHE.get("warm"):
        # first execution in a process runs ~15-20% slower (cold NEFF/DMA/
        # clock state); one throwaway execution warms the device
        run_bass_kernel_spmd(nc, in_maps, core_ids=list(range(N_CORES)))
        _PROG_CACHE["warm"] = True
    res = run_bass_kernel_spmd(nc, in_maps, core_ids=list(range(N_CORES)))
    out = np.empty((B, H, W, FOUT), np.float32)
    for b in range(N_CORES):
        out[b] = res.results[b]["out"].T.reshape(H, W, FOUT)
    return out



# revision 16
# speedup vs baseline: 1.2134x; 1.2134x over previous
"""Attention-Augmented Conv2D (AAConv2D) distributed Bass kernel for 8 TRN2 NeuronCores.

Strategy: pure data-parallel over batch (B=8 -> one image per core, weights
replicated, zero collectives). Per core, for one [32,32,256] image:

  conv branch : 3x3 SAME conv (256->256ch) as 9 shifted-window matmuls
                accumulated in PSUM, one tap per attention chunk-slot
                (PE filler under the ACT-bound softmax stream).
  attn branch : kqv 1x1 conv (channel-major k/q, position-major v),
                per-head S^T = K Q^T computed with an AUGMENTED contraction
                (32 qk dims + 64 one-hot w/h-offset dims = 96) so the
                relative-position logits ride in the same matmul stream;
                exp on ScalarE (max-free softmax, logits are O(10) so fp32
                exp is safe); P^T V via matmul with [v|1] stationary
                (sumexp ride-along); per-head normalize on VectorE/GpSimd;
                output projection at the tail.

Schedule notes (v3 — rebuilt from the v2 trace):
  - v2 spent 53us before the first exp and had ~4us inter-head ACT
    bubbles (conv+rel matmuls queued between heads) plus 92us of HAM
    half-clock. v3 pipelines one global chunk stream: per chunk slot the
    PE does [S(next), PV(prev), 1 conv tap] (~1.05us) under the 1.11us
    exp, with no inter-head break in the rotation.
  - All rel-logit matmuls run upfront in two 4-head-concurrent waves,
    reading q directly from the kqv-escape strips (krw4/krh4 tables are
    strip-replicated host-side) — the v2 qrep replication DMAs are gone.
  - The one-hot delta rows live in two parity stationary tiles st0/st1
    (rows 32:96 written once from DRAM); only the 32 k-rows are
    re-DMA'd per head, two heads ahead, on the sync queue.
  - The scalar queue carries NO DMAs after startup (v2 put 26us of
    descriptor time on it, starving exp issue); exp's ACT table is
    preloaded by a dummy activation at t~0.
  - Input DMAs fan across sync/vector/scalar/gpsimd queues in need
    order (xtc+wkqv first).
  - PSUM (8 banks): S rotation 2x[128,1024]f32 (4) + PV accumulator
    [128,1024]f32 (2) + conv/startup/misc tile (2).

All heavy matmuls in bf16 (fp32 matmul is 4x slower on TRN2 PE).
Host does layout-only prep: batch sharding, transposes to channel-major,
bf16 casts, relative-table window expansion, one-hot delta tables, and the
exact algebraic fold of the v-bias into the projection bias.
"""

import contextlib

import numpy as np
import ml_dtypes

BF16 = ml_dtypes.bfloat16

B, H, W, FIN = 8, 32, 32, 256
POS = H * W
FOUT, K, DK, DV, NH = 512, 3, 256, 256, 8
DKH, DVH = DK // NH, DV // NH
FOUT_CONV = FOUT - DV  # 256
N_CORES = 8

_PROG_CACHE = {}


def _build_program(variant="full"):
    """Build (and cache) the compiled Bass program. Same program for all 8
    cores (SPMD); per-core data arrives via the per-core input maps."""
    if ("nc", variant) in _PROG_CACHE:
        return _PROG_CACHE[("nc", variant)]

    import concourse.bass as bass
    import concourse.bacc as bacc
    import concourse.tile as tile
    from concourse import mybir

    BF = mybir.dt.bfloat16
    F32 = mybir.dt.float32
    EXP = mybir.ActivationFunctionType.Exp

    nc = bacc.Bacc("TRN2", target_bir_lowering=False, debug=False,
                   num_devices=N_CORES)

    # ---- DRAM parameters ----
    PADW = H + 2  # 34
    xpad_d = nc.dram_tensor("xpad", [FIN, PADW * PADW], BF, kind="ExternalInput")
    xtc_d = nc.dram_tensor("xtc", [FIN, POS], BF, kind="ExternalInput")
    wkqv_d = nc.dram_tensor("wkqv", [FIN, 2 * DK + DV], BF, kind="ExternalInput")
    wconv_d = nc.dram_tensor("wconv", [2, 128, K * K * FOUT_CONV], BF,
                             kind="ExternalInput")
    wproj_d = nc.dram_tensor("wproj", [DV, DV], BF, kind="ExternalInput")
    biases_d = nc.dram_tensor("biases", [128, 8], F32, kind="ExternalInput")
    krw_d = nc.dram_tensor("krw4", [128, 1024], BF, kind="ExternalInput")
    krh_d = nc.dram_tensor("krh4", [128, 1024], BF, kind="ExternalInput")
    delta_d = nc.dram_tensor("delta", [2 * W, POS], BF, kind="ExternalInput")
    out_d = nc.dram_tensor("out", [FOUT, POS], BF, kind="ExternalOutput")
    dbg_d = None
    if variant.startswith("debug"):
        dbg_d = nc.dram_tensor("dbg", [96, NH * POS], BF, kind="ExternalOutput")
        dbg2_d = nc.dram_tensor("dbg2", [2, 128, POS], BF, kind="ExternalOutput")
        dbg3_d = nc.dram_tensor("dbg3", [2, 96, POS], BF, kind="ExternalOutput")
        dbg4_d = nc.dram_tensor("dbg4", [8, 128, POS], BF, kind="ExternalOutput")
        dbg5_d = nc.dram_tensor("dbg5", [8, DVH + 1, POS], BF, kind="ExternalOutput")

    xpad_in = xpad_d.ap()
    xtc = xtc_d.ap()
    wkqv = wkqv_d.ap()
    wconv = wconv_d.ap()
    wproj = wproj_d.ap()
    biases = biases_d.ap()
    krw = krw_d.ap()
    krh = krh_d.ap()
    delta = delta_d.ap()
    out = out_d.ap()

    with tile.TileContext(nc) as tc, contextlib.ExitStack() as ctx:
        consts = ctx.enter_context(tc.tile_pool(name="consts", bufs=1))
        xpads = ctx.enter_context(tc.tile_pool(name="xpads", bufs=1))
        kqpool = ctx.enter_context(tc.tile_pool(name="kqpool", bufs=1))
        vopool = ctx.enter_context(tc.tile_pool(name="vopool", bufs=1))
        attall = ctx.enter_context(tc.tile_pool(name="attall", bufs=1))
        megas = ctx.enter_context(tc.tile_pool(name="megas", bufs=1))
        stp = ctx.enter_context(tc.tile_pool(name="stp", bufs=1))
        cacc = ctx.enter_context(tc.tile_pool(name="cacc", bufs=1))
        work = ctx.enter_context(tc.tile_pool(name="work", bufs=4))
        small = ctx.enter_context(tc.tile_pool(name="small", bufs=4))
        outp = ctx.enter_context(tc.tile_pool(name="outp", bufs=2))
        # PSUM: 8 banks = ps_s 2x[128,1024]f32 (S chunks; startup kqv/rel)
        # + ps_at 1x (PV accumulator) + ps_w 1x (conv groups, prewarm, v,
        # tail proj/rcp-broadcast).
        ps_s = ctx.enter_context(tc.tile_pool(name="ps_s", bufs=2,
                                              space=bass.MemorySpace.PSUM))
        ps_at = ctx.enter_context(tc.tile_pool(name="ps_at", bufs=1,
                                               space=bass.MemorySpace.PSUM))
        ps_w = ctx.enter_context(tc.tile_pool(name="ps_w", bufs=1,
                                              space=bass.MemorySpace.PSUM))

        # ---- startup input DMAs, fanned across the 3 DMA-capable queues
        # (sync/scalar/gpsimd) in need order ----
        # sync:   xtc0, xtc1, biases, krw, delta->st0, xpad0, row copies, wproj
        # scalar: wkqv0, krh, wconv0                 (clean before first exp)
        # gpsimd: wkqv1, delta->st1, wconv1, xpad1, q-row copies h4-7
        xt_sb = []
        for f in range(2):
            t = xpads.tile([128, POS], BF, tag=f"xtsb{f}")
            nc.sync.dma_start(out=t[:], in_=xtc[f * 128:(f + 1) * 128, :])
            xt_sb.append(t)
        wkqv_sb = []
        for f in range(2):
            t = consts.tile([128, 2 * DK + DV], BF, tag=f"wkqv{f}")
            eng = nc.scalar if f == 0 else nc.gpsimd
            eng.dma_start(out=t[:], in_=wkqv[f * 128:(f + 1) * 128, :])
            wkqv_sb.append(t)
        ball_sb = consts.tile([128, 8], F32, tag="ball")
        nc.sync.dma_start(out=ball_sb[:], in_=biases[:, :])
        bkq_sb = [ball_sb[:, cc:cc + 1] for cc in range(4)]
        bconv_sb = [ball_sb[:, 4 + co:5 + co] for co in range(2)]
        bproj_sb = [ball_sb[:, 6 + co:7 + co] for co in range(2)]
        krw_sb = consts.tile([128, 1024], BF, tag="krw")
        nc.sync.dma_start(out=krw_sb[:], in_=krw[:, :])
        krh_sb = consts.tile([128, 1024], BF, tag="krh")
        nc.scalar.dma_start(out=krh_sb[:], in_=krh[:, :])

        # parity stationary tiles for S: rows 0:32 = k of the running head
        # (rewritten two heads ahead), rows 32:96 = one-hot deltas (static)
        st = []
        for i in range(2):
            t = stp.tile([96, POS], BF, tag=f"st{i}")
            eng = nc.sync if i == 0 else nc.gpsimd
            eng.dma_start(out=t[32:96, :], in_=delta[:, :])
            st.append(t)

        wconv_sb = []
        for f in range(2):
            t = consts.tile([128, K * K * FOUT_CONV], BF, tag=f"wconv{f}")
            eng = nc.scalar if f == 0 else nc.gpsimd
            eng.dma_start(out=t[:], in_=wconv[f, :, :])
            wconv_sb.append(t)
        xpad = []
        for f in range(2):
            t = xpads.tile([128, PADW * PADW], BF, tag=f"xpad{f}")
            eng = nc.sync if f == 0 else nc.gpsimd
            eng.dma_start(out=t[:], in_=xpad_in[f * 128:(f + 1) * 128, :])
            xpad.append(t)
        wproj_sb = []
        for f in range(2):
            t = consts.tile([128, DV], BF, tag=f"wproj{f}")
            nc.sync.dma_start(out=t[:], in_=wproj[f * 128:(f + 1) * 128, :])
            wproj_sb.append(t)

        # per-head q/rel rows for the S moving operand:
        # rows 0:32 q (DMA'd from the kqv escape), 32:64 rel_w, 64:96 rel_h
        rhmega = megas.tile([96, NH * POS], BF, tag="rhmega")
        rh3 = rhmega.rearrange("p (h c) -> p h c", h=NH)

        ones_sb = consts.tile([1, 32], BF, tag="ones")
        nc.vector.memset(ones_sb[:], 1.0)

        # ---- PE pre-warm (memset-fed) bridges the input-DMA wait so the
        # HAM clock-gate is released before the first real matmul ----
        wu = consts.tile([128, 512], BF, tag="wu")
        nc.vector.memset(wu[:], 0.25)
        wups = ps_w.tile([128, POS], F32, tag="pw", name="wups")
        for i in range(6):
            nc.tensor.matmul(wups[:, 0:512], lhsT=wu[:, 0:128], rhs=wu[:],
                             start=True, stop=True)
        # preload the exp ACT table (~2.7us) long before the first real exp
        wrm = small.tile([1, 32], BF, tag="wrm", name="wrm")
        nc.scalar.activation(out=wrm[:], in_=wu[0:1, 0:32], func=EXP)

        # ---- kqv: k and q sections, channel-major [co, pos] ----
        # order (q0, k0, q1, k1); after each escape, per-head q-row DMAs
        # (q -> rhmega rows 0:32) / k-row DMAs (k of h0/h1 -> st parity)
        kq_sb = [None] * 4
        for cc in (2, 0, 3, 1):
            ps = ps_s.tile([128, POS], F32, tag="ps", name=f"kqps{cc}")
            for f in range(2):
                for nh in range(2):
                    nc.tensor.matmul(
                        ps[:, nh * 512:(nh + 1) * 512],
                        lhsT=wkqv_sb[f][:, cc * 128:(cc + 1) * 128],
                        rhs=xt_sb[f][:, nh * 512:(nh + 1) * 512],
                        start=(f == 0), stop=(f == 1))
            t = kqpool.tile([128, POS], BF, tag=f"kq{cc}", name=f"kq{cc}")
            nc.vector.tensor_scalar_add(out=t[:], in0=ps[:], scalar1=bkq_sb[cc][:])
            kq_sb[cc] = t
            sec = cc % 2  # head group of this section (0: h0-3, 1: h4-7)
            if cc >= 2:  # q section: q rows into the per-head mega blocks
                for hh in range(4):
                    h = 4 * sec + hh
                    eng = nc.sync if sec == 0 else nc.gpsimd
                    eng.dma_start(out=rh3[0:32, h, :],
                                  in_=t[32 * hh:32 * hh + 32, :])
            elif cc == 0:  # k section h0-3: prime the two parity tiles
                for h in range(2):
                    nc.sync.dma_start(out=st[h][0:32, :],
                                      in_=t[32 * h:32 * h + 32, :])

        # ---- rel-logit matmuls, two 4-head-concurrent waves ----
        # Head h (strip s=h%4) reads q directly from its kqv-escape strip;
        # krw4/krh4 are strip-replicated host-side. rel_w for (wq, w')
        # lands at psum[32s + w', 32*wq + hq]; rel_h for (hq, h') at
        # psum[32s + h', 32*hq + wq] (contiguous).
        def rel_wave(w, pw, phh):
            qsec = kq_sb[2 + w]
            q3 = qsec.rearrange("p (b a) -> p a b", a=W)  # [p, wq, hq]
            for a in range(W):
                for s in range(4):
                    nc.tensor.matmul(
                        pw[32 * s:32 * s + 32, 32 * a:32 * a + 32],
                        lhsT=krw_sb[32 * s:32 * s + 32, a * 32:(a + 1) * 32],
                        rhs=q3[32 * s:32 * s + 32, a, :],
                        start=True, stop=True, tile_position=(32 * s, 32 * s))
                for s in range(4):
                    nc.tensor.matmul(
                        phh[32 * s:32 * s + 32, 32 * a:32 * a + 32],
                        lhsT=krh_sb[32 * s:32 * s + 32, a * 32:(a + 1) * 32],
                        rhs=qsec[32 * s:32 * s + 32, a * 32:(a + 1) * 32],
                        start=True, stop=True, tile_position=(32 * s, 32 * s))

        def rel_escape(w, pw, phh):
            for s in range(4):
                h = 4 * w + s
                # un-permute rel_w: src col (wq, hq) -> dst col (hq, wq)
                src = pw[32 * s:32 * s + 32, :].rearrange(
                    "p (a b) -> p b a", a=W)
                nc.vector.tensor_copy(out=rh3[32:64, h, :].rearrange(
                    "p (b a) -> p b a", a=W), in_=src)
                nc.vector.tensor_copy(out=rh3[64:96, h, :],
                                      in_=phh[32 * s:32 * s + 32, :])

        pw0 = ps_s.tile([128, POS], F32, tag="ps", name="relw0")
        phh0 = ps_s.tile([128, POS], F32, tag="ps", name="relh0")
        rel_wave(0, pw0, phh0)
        rel_escape(0, pw0, phh0)
        pw1 = ps_s.tile([128, POS], F32, tag="ps", name="relw1")
        phh1 = ps_s.tile([128, POS], F32, tag="ps", name="relh1")
        rel_wave(1, pw1, phh1)
        rel_escape(1, pw1, phh1)

        # ---- v: position-major [pos, dv] -> vomega with ones interleave ----
        vomega = vopool.tile([128, 8 * NH * (DVH + 1)], BF, tag="vomega")
        vom4 = vomega.rearrange("p (k h d) -> p k h d", k=8, d=DVH + 1)
        nc.vector.memset(vom4[:, :, :, DVH:DVH + 1], 1.0)
        for half in range(2):
            ps = ps_w.tile([128, POS], F32, tag="pw", name=f"vps{half}")
            for q in range(4):
                kc = half * 4 + q
                for f in range(2):
                    nc.tensor.matmul(
                        ps[:, q * 256:(q + 1) * 256],
                        lhsT=xt_sb[f][:, kc * 128:(kc + 1) * 128],
                        rhs=wkqv_sb[f][:, 2 * DK:2 * DK + DV],
                        start=(f == 0), stop=(f == 1))
            nc.vector.tensor_copy(
                out=vom4[:, half * 4:(half + 1) * 4, :, 0:DVH],
                in_=ps.rearrange("p (k h d) -> p k h d", k=4, d=DVH))

        att_all = []
        for f in range(2):
            t = attall.tile([128, POS], BF, tag=f"att{f}", name=f"att{f}")
            att_all.append(t)
        # conv fp32 SBUF accumulators (per co half; each (nh, f) group's
        # 9-tap psum result is folded in with a DVE copy/add)
        conv_acc = []
        for co in range(2):
            t = cacc.tile([128, POS], F32, tag=f"cacc{co}")
            conv_acc.append(t)

        def xwin(f, dy, dx, h0, hn):
            # [128, hn, 32] window of the padded image
            t3 = xpad[f].rearrange("p (a b) -> p a b", a=PADW)
            return t3[:, h0 + dy:h0 + dy + hn, dx:dx + W]

        def conv_escape(co):
            if variant == "debug_noconv":
                return
            ot = outp.tile([128, POS], BF, tag="out", name=f"cot{co}")
            nc.vector.tensor_scalar_add(out=ot[:], in0=conv_acc[co][:],
                                        scalar1=bconv_sb[co][:])
            nc.sync.dma_start(out=out[co * 128:(co + 1) * 128, :], in_=ot[:])

        # ---- per-head attention, one global pipelined chunk stream ----
        def inner_head(h):
            at = ps_at.tile([128, POS], F32, tag="at", name=f"at{h}")
            psb_t = [None] * 8
            # conv group h rides this head's chunk slots: one 512-col tap
            # after each s_step (9 taps + fold across the 8 slots)
            co, nh, f = h // 4, (h // 2) % 2, h % 2
            cps = ps_w.tile([128, POS], F32, tag="pw", name=f"cps{h}")

            def conv_tap(tp):
                if variant == "debug_noconv":
                    return
                dy, dx = tp // 3, tp % 3
                o0 = tp * FOUT_CONV + co * 128
                nc.tensor.matmul(
                    cps[:, nh * 512:(nh + 1) * 512],
                    lhsT=wconv_sb[f][:, o0:o0 + 128],
                    rhs=xwin(f, dy, dx, nh * 16, 16),
                    start=(tp == 0), stop=(tp == 8))

            def conv_fold():
                if variant == "debug_noconv":
                    return
                acc = conv_acc[co][:, nh * 512:(nh + 1) * 512]
                src = cps[:, nh * 512:(nh + 1) * 512]
                if f == 0:
                    nc.vector.tensor_copy(out=acc, in_=src)
                else:
                    nc.vector.tensor_add(acc, acc, src)

            def s_step(kc):
                sps = ps_s.tile([128, POS], F32, tag="ps", name=f"sps{h}_{kc}")
                for nh2 in range(2):
                    nc.tensor.matmul(
                        sps[:, nh2 * 512:(nh2 + 1) * 512],
                        lhsT=st[h % 2][0:96, kc * 128:(kc + 1) * 128],
                        rhs=rh3[0:96, h, nh2 * 512:(nh2 + 1) * 512],
                        start=True, stop=True)
                psb = work.tile([128, POS], BF, tag="pexp", name=f"psb{h}_{kc}")
                nc.scalar.activation(out=psb[:], in_=sps[:], func=EXP)
                psb_t[kc] = psb
                if dbg_d is not None and h == 0:
                    nc.sync.dma_start(out=dbg4_d.ap()[kc, :, :], in_=psb[:])

            def pv_step(kc):
                for nh2 in range(2):
                    nc.tensor.matmul(
                        at[0:DVH + 1, nh2 * 512:(nh2 + 1) * 512],
                        lhsT=vomega[:, kc * NH * (DVH + 1) + h * (DVH + 1):
                                    kc * NH * (DVH + 1) + (h + 1) * (DVH + 1)],
                        rhs=psb_t[kc][:, nh2 * 512:(nh2 + 1) * 512],
                        start=(kc == 0), stop=(kc == 7))

            s_step(0)
            conv_tap(0)
            s_step(1)
            conv_tap(1)
            pv_step(0)
            for kc in range(2, 8):
                s_step(kc)
                conv_tap(kc)
                pv_step(kc - 1)
            conv_tap(8)
            pv_step(7)
            conv_fold()

            # k rows of head h+2 into this head's parity tile (overlaps
            # head h+1; Tile orders it after this head's last S read)
            if h + 2 < 8:
                h2 = h + 2
                nc.sync.dma_start(
                    out=st[h % 2][0:32, :],
                    in_=kq_sb[0 if h2 < 4 else 1][32 * (h2 % 4):
                                                  32 * (h2 % 4) + 32, :])

            # psum-escape copy (frees rows 0:33 for the next head), then
            # normalize: attn_h = (P^T V)[0:32] / sumexp (row 32)
            cmb = small.tile([DVH + 1, POS], BF, tag="cmb", name=f"cmb{h}")
            nc.vector.tensor_copy(out=cmb[:], in_=at[0:DVH + 1, :])
            if dbg_d is not None:
                nc.sync.dma_start(out=dbg5_d.ap()[h, :, :], in_=cmb[:])
            sec = h // 4
            g = (h % 4) * 32
            s8 = small.tile([128, 8], BF, tag="s8", name=f"s8{h}")
            nc.gpsimd.dma_start(out=s8[:], in_=cmb[DVH:DVH + 1, :])
            rcp8 = small.tile([128, 8], BF, tag="rcp8", name=f"rcp8{h}")
            with nc.allow_low_precision(reason="1/sumexp in bf16 is within "
                                        "the softmax rounding budget"):
                nc.vector.reciprocal(out=rcp8[:], in_=s8[:])
            rcpf = small.tile([1, POS], BF, tag="rcpf", name=f"rcpf{h}")
            nc.sync.dma_start(out=rcpf[:], in_=rcp8[:])
            an = small.tile([32, POS], BF, tag="an", name=f"an{h}")
            if h == 7:
                # tail: broadcast 1/sumexp via a K=1 PE matmul (PE is idle
                # here and this cuts two GpSimd queue hops off the tail)
                rps = ps_w.tile([128, POS], F32, tag="pw", name="rcppe")
                for nh2 in range(2):
                    nc.tensor.matmul(
                        rps[0:32, nh2 * 512:(nh2 + 1) * 512],
                        lhsT=ones_sb[:, :],
                        rhs=rcpf[:, nh2 * 512:(nh2 + 1) * 512],
                        start=True, stop=True)
                nc.vector.tensor_mul(an[:], cmb[0:DVH, :], rps[0:32, :])
            else:
                rcpb = small.tile([32, POS], BF, tag="rcpb", name=f"rcpb{h}")
                nc.gpsimd.partition_broadcast(rcpb[:], rcpf[:])
                nc.vector.tensor_mul(an[:], cmb[0:DVH, :], rcpb[:])
            nc.gpsimd.dma_start(out=att_all[sec][g:g + 32, :], in_=an[:])

        for h in range(8):
            inner_head(h)
            if h == 4:
                conv_escape(0)
        conv_escape(1)
        if dbg_d is not None:
            nc.sync.dma_start(out=dbg_d.ap()[:, :], in_=rhmega[:, :])
            for f in range(2):
                nc.sync.dma_start(out=dbg2_d.ap()[f, :, :], in_=att_all[f][:])
                nc.sync.dma_start(out=dbg3_d.ap()[f, :, :], in_=st[f][:])

        # ---- tail: output projection (f halves accumulate in psum) ----
        for co in range(2):
            pool = ps_s if co == 0 else ps_w
            ps = pool.tile([128, POS], F32, tag="ps" if co == 0 else "pw",
                           name=f"pps{co}")
            for f in range(2):
                for nh2 in range(2):
                    nc.tensor.matmul(
                        ps[:, nh2 * 512:(nh2 + 1) * 512],
                        lhsT=wproj_sb[f][:, co * 128:(co + 1) * 128],
                        rhs=att_all[f][:, nh2 * 512:(nh2 + 1) * 512],
                        start=(f == 0), stop=(f == 1))
            ot = outp.tile([128, POS], BF, tag="out", name=f"pot{co}")
            nc.vector.tensor_scalar_add(out=ot[:], in0=ps[:],
                                        scalar1=bproj_sb[co][:])
            nc.sync.dma_start(
                out=out[FOUT_CONV + co * 128:FOUT_CONV + (co + 1) * 128, :],
                in_=ot[:])

    nc.compile()
    _PROG_CACHE[("nc", variant)] = nc
    return nc


def _host_prep(x, w_kqv, b_kqv, w_proj, b_proj, w_conv, b_conv,
               key_rel_w, key_rel_h):
    """Layout-only host prep -> per-core input maps."""
    x = np.asarray(x, np.float32)
    w_kqv = np.asarray(w_kqv, np.float32)
    b_kqv = np.asarray(b_kqv, np.float32)
    w_proj = np.asarray(w_proj, np.float32)
    b_proj = np.asarray(b_proj, np.float32)
    w_conv = np.asarray(w_conv, np.float32)
    b_conv = np.asarray(b_conv, np.float32)
    key_rel_w = np.asarray(key_rel_w, np.float32)
    key_rel_h = np.asarray(key_rel_h, np.float32)

    scale = np.float32(DKH ** -0.5)
    wkqv = w_kqv.copy()
    wkqv[:, DK:2 * DK] *= scale           # fold q scaling into the weights
    bkq = b_kqv[:2 * DK].copy()
    bkq[DK:] *= scale
    # fold the v bias through the projection: attn = (attn0 + bv) Wp + bp
    bproj_eff = b_proj + b_kqv[2 * DK:] @ w_proj
    # combined per-partition bias tile [128, 8]:
    # cols 0-3 = b_kq 128-chunks, 4-5 = b_conv chunks, 6-7 = b_proj chunks
    ball = np.stack([bkq[0:128], bkq[128:256], bkq[256:384], bkq[384:512],
                     b_conv[0:128], b_conv[128:256],
                     bproj_eff[0:128], bproj_eff[128:256]], axis=1)

    # window-expanded relative tables, replicated to all 4 partition groups:
    #   krw4[32r + d, wq*32 + w'] = key_rel_w[w' - wq + 31, d]
    idx = (np.arange(W)[None, :] - np.arange(W)[:, None] + (W - 1))  # [wq, w']
    krw = key_rel_w[idx]                   # [wq, w', 32]
    krw4 = np.tile(krw.transpose(2, 0, 1).reshape(DKH, W * W), (4, 1))
    krh = key_rel_h[idx]
    krh4 = np.tile(krh.transpose(2, 0, 1).reshape(DKH, H * H), (4, 1))

    # one-hot offset deltas: rows 0-31 wk one-hots, rows 32-63 hk one-hots
    kpos = np.arange(POS)
    deltas = np.zeros((2 * W, POS), np.float32)
    deltas[kpos % W, kpos] = 1.0
    deltas[W + kpos // W, kpos] = 1.0

    # conv weights repacked so each 128-channel chunk's 9 taps are one
    # contiguous per-partition run: wconv[f][p, tp*256 + o]
    wc = w_conv.reshape(K * K, 2, 128, FOUT_CONV)          # [tap, f, p, o]
    wc = np.ascontiguousarray(wc.transpose(1, 2, 0, 3)).reshape(
        2, 128, K * K * FOUT_CONV)

    shared = {
        "wkqv": wkqv.astype(BF16),
        "wconv": wc.astype(BF16),
        "wproj": w_proj.astype(BF16),
        "biases": ball.astype(np.float32),
        "krw4": krw4.astype(BF16),
        "krh4": krh4.astype(BF16),
        "delta": deltas.astype(BF16),
    }
    PADW = H + 2
    in_maps = []
    for b in range(N_CORES):
        m = dict(shared)
        xt = np.ascontiguousarray(x[b].reshape(POS, FIN).T)   # [FIN, POS]
        xp = np.zeros((FIN, PADW, PADW), np.float32)
        xp[:, 1:H + 1, 1:W + 1] = xt.reshape(FIN, H, W)
        m["xpad"] = xp.reshape(FIN, PADW * PADW).astype(BF16)
        m["xtc"] = xt.astype(BF16)
        in_maps.append(m)
    return in_maps


def kernel(x, w_kqv, b_kqv, w_proj, b_proj, w_conv, b_conv,
           key_rel_w, key_rel_h):
    from concourse.bass_utils import run_bass_kernel_spmd

    nc = _build_program()
    in_maps = _host_prep(x, w_kqv, b_kqv, w_proj, b_proj, w_conv, b_conv,
                         key_rel_w, key_rel_h)
    if not _PROG_CACHE.get("warm"):
        # first execution in a process runs ~15-20% slower (cold NEFF/DMA/
        # clock state); one throwaway execution warms the device
        run_bass_kernel_spmd(nc, in_maps, core_ids=list(range(N_CORES)))
        _PROG_CACHE["warm"] = True
    res = run_bass_kernel_spmd(nc, in_maps, core_ids=list(range(N_CORES)))
    out = np.empty((B, H, W, FOUT), np.float32)
    for b in range(N_CORES):
        out[b] = res.results[b]["out"].T.reshape(H, W, FOUT)
    return out
